# revision 3
# baseline (speedup 1.0000x reference)
"""Trainium2 Bass kernel for per-sample-routed ConvTranspose1d (Dereverb T60
decoder), fp16 edition.

Math per sample b (routed weight W (Cin=512, K=16), stride 8, pad 8):
    A[k, q] = sum_ci W[ci, k] * x[ci, q]      (16x512 @ 512x4000 matmul)
    y[8m+p] = A[p, m+1] + A[p+8, m]           (overlap-add, m in [0, 3999))

Sharding: pure data parallel, B=16 -> 2 samples on each of 8 NeuronCores.
Routing (t60 -> 1 of 41 kernels) is a host-side gather.

The whole pipeline is HBM-bound on loading x, so x/w are sent as fp16
(tolerance 2e-2 vs ~1e-3 fp16 error) halving DMA bytes, and fp16 matmuls
run 4x faster than fp32 on the PE. Output y is written to HBM as a dense
fp16 [128, 256] tile per sample in transposed layout; the host undoes the
permutation (free) instead of the device doing 32B-line scatter DMA.

Device kernel per core, per sample:
  - lhsT "w40" (128, 4 chunks, 40): taps 0..7 at cols 0..7, taps 8..15 at
    cols 32..39 (host pre-packs; engine ops need partition bases in
    {0,32,64,96} so psum hi rows live at 32:40).
  - 8 j-tiles: psum_A (40, 512) accumulates 4 K-chunk fp16 matmuls.
  - ACT copies lo rows to SBUF alo (8, 4000) fp32; DVE adds
    alo[:, m+1] + psum_hi[:, m] -> z128 fp16, which stacks 4 blocks of
    1024 m's at partition bases {0,32,64,96}. The add for block j runs
    right after the j+1 copy (one column straddles), freeing psum banks.
  - 8 PE transposes (128,128 fp16) -> psum_t[i, 32a+p] = y[8*(1024a+128k+i)+p];
    DVE copies valid cols into ysb (128, 4a, 8k, 8p) fp16; one dense DMA
    per sample to HBM.
"""
import numpy as np

import concourse.bass as bass
import concourse.tile as tile
from concourse import bacc, mybir
from concourse.bass_utils import run_bass_kernel_spmd
from concourse.masks import make_identity

B, CIN, L, KSZ = 16, 512, 4000, 16
LOUT = (L - 1) * 8 - 2 * 8 + KSZ  # 31992
NCORES = 8
PER = B // NCORES                 # 2 samples per core
NCHUNK = CIN // 128               # 4
JW = 512
NJ = 8                            # ceil(4000/512)
MV = L - 1                        # 3999 valid output m positions
XSL = 2048                        # x DMA slice width (cols)
F32 = mybir.dt.float32
F16 = mybir.dt.float16

_CACHE = {}


def _build(reps=1, xbufs=3, pabufs=4, ptbufs=4, zbufs=2):
    nc = bacc.Bacc("TRN2", target_bir_lowering=False, debug=False,
                   num_devices=NCORES)
    x = nc.dram_tensor("x", [PER, CIN, L], F16, kind="ExternalInput").ap()
    w = nc.dram_tensor("w", [PER, 128, NCHUNK * 40], F16,
                       kind="ExternalInput").ap()
    y = nc.dram_tensor("y", [PER, 128, 256], F16, kind="ExternalOutput").ap()

    with tile.TileContext(nc) as tc:
        with tc.tile_pool(name="xp", bufs=xbufs) as xp, \
             tc.tile_pool(name="wp", bufs=2) as wp, \
             tc.tile_pool(name="alop", bufs=2) as alop, \
             tc.tile_pool(name="zp", bufs=zbufs) as zp, \
             tc.tile_pool(name="yp", bufs=2) as yp, \
             tc.tile_pool(name="cst", bufs=1) as cst, \
             tc.tile_pool(name="pa", bufs=pabufs, space="PSUM") as pa, \
             tc.tile_pool(name="pt", bufs=ptbufs, space="PSUM") as pt:

            ident = cst.tile([128, 128], F16)
            make_identity(nc, ident[:])

            for rep in range(reps):
                for s in range(PER):
                    w40 = wp.tile([128, NCHUNK, 40], F16, tag="w40")
                    nc.scalar.dma_start(
                        w40[:], w[s].rearrange("p (c k) -> p c k", c=NCHUNK))

                    alo = alop.tile([8, L], F32, tag="alo")
                    z128 = zp.tile([128, 1024], F16, tag="z128")
                    psums = {}

                    def add_piece(j):
                        # z128[32a+p, 512h+c] = A[p, 512j+c+1] + A[p+8, 512j+c]
                        a, h = j // 2, j % 2
                        m0 = JW * j
                        nz = min(JW, MV - m0)  # 512, last piece 415
                        nc.vector.tensor_tensor(
                            z128[32 * a: 32 * a + 8, 512 * h: 512 * h + nz],
                            alo[0:8, m0 + 1: m0 + 1 + nz],
                            psums[j][32:40, 0:nz],
                            mybir.AluOpType.add)

                    for j in range(NJ):
                        n = min(JW, L - JW * j)  # 512 or 416
                        j0 = JW * j
                        if j0 % XSL == 0:
                            a0 = j0
                            wa = min(XSL, L - a0)
                            xt = xp.tile([128, NCHUNK, XSL], F16, tag="xt")
                            nc.sync.dma_start(
                                xt[:, :, 0:wa],
                                x[s].rearrange("(c p) l -> p c l", p=128)
                                   [:, :, a0:a0 + wa])
                        ps = pa.tile([40, JW], F32, tag="pa")
                        psums[j] = ps
                        for c in range(NCHUNK):
                            nc.tensor.matmul(
                                ps[:, 0:n], w40[:, c, :],
                                xt[:, c, j0 - a0: j0 - a0 + n],
                                start=(c == 0), stop=(c == NCHUNK - 1))
                        nc.scalar.copy(alo[:, j0: j0 + n], ps[0:8, 0:n])
                        if j >= 1:
                            add_piece(j - 1)
                    add_piece(NJ - 1)

                    ysb = yp.tile([128, 4, 8, 8], F16, tag="ysb")
                    for k in range(8):
                        tps = pt.tile([128, 128], F16, tag="pt")
                        nc.tensor.matmul(tps[:],
                                         z128[:, 128 * k: 128 * (k + 1)],
                                         ident[:], is_transpose=True,
                                         start=True, stop=True)
                        nc.vector.tensor_copy(
                            ysb[:, :, k, :],
                            tps[:].rearrange("j (a q) -> j a q", a=4)
                               [:, :, 0:8])

                    nc.scalar.dma_start(
                        y[s], ysb[:].rearrange("j a k p -> j (a k p)"))

    nc.compile()
    return nc


def get_nc(reps=1, f32r=False):
    key = reps
    if key not in _CACHE:
        _CACHE[key] = _build(reps=reps)
    return _CACHE[key]


def _route(t60s):
    idx = np.round(t60s.astype(np.float32) * np.float32(100.0))
    return np.tile(idx.astype(np.int32), 2) - 10  # (B,)


def make_in_maps(input, t60s, kernel_weight):
    idx = _route(np.asarray(t60s))
    wg = np.asarray(kernel_weight)[idx, :, 0, :]          # (B, Cin, K) f32
    # pack to lhsT layout: [B, 128p, 4c, 40], taps 0..7 -> cols 0..7,
    # taps 8..15 -> cols 32..39 (cin = c*128 + p)
    wr = wg.reshape(B, NCHUNK, 128, KSZ).transpose(0, 2, 1, 3)
    w40 = np.zeros((B, 128, NCHUNK, 40), dtype=np.float16)
    w40[:, :, :, 0:8] = wr[:, :, :, 0:8]
    w40[:, :, :, 32:40] = wr[:, :, :, 8:16]
    w40 = w40.reshape(B, 128, NCHUNK * 40)
    xin = np.asarray(input).astype(np.float16)
    in_maps = []
    for c in range(NCORES):
        sl = slice(PER * c, PER * (c + 1))
        in_maps.append({
            "x": np.ascontiguousarray(xin[sl]),
            "w": np.ascontiguousarray(w40[sl]),
        })
    return in_maps


def _unpack_y(yraw):
    # yraw: (PER, 128, 256) fp16; ysb[j, a, k, p] = y[8*(1024a+128k+j)+p]
    yr = yraw.astype(np.float32).reshape(PER, 128, 4, 8, 8)
    return yr.transpose(0, 2, 3, 1, 4).reshape(PER, 4096 * 8)[:, :LOUT]


def _run(input, t60s, kernel_weight, trace=False):
    nc = get_nc()
    in_maps = make_in_maps(input, t60s, kernel_weight)
    res = run_bass_kernel_spmd(nc, in_maps, core_ids=list(range(NCORES)),
                               trace=trace)
    out = np.empty((B, 1, LOUT), dtype=np.float32)
    for c in range(NCORES):
        out[PER * c: PER * (c + 1), 0, :] = _unpack_y(res.results[c]["y"])
    return out, res


def kernel(input, t60s, kernel_weight):
    out, _ = _run(input, t60s, kernel_weight, trace=False)
    return out
